# revision 20
# baseline (speedup 1.0000x reference)
"""Chamfer distance kernel for 8 trn2 NeuronCores.

Sharding: data-parallel over batch B=4 (2 cores per batch element), with the
N=8192 predicted-point axis split in half across the core pair. Each core
computes, for its (batch, n-half):
  - d2[n, m] squared-distance tiles on the TensorEngine via a K=16 fp16
    hi/lo-split matmul folding |p|^2 + |t|^2 - 2 p.t into one contraction.
    PSUM accumulates fp32; 4 groups of 2048 per 128-row n-tile.
  - ACT (scalar) drains PSUM -> fp16 c for m in [0, DSPLIT).
  - DVE drains the [DSPLIT, M) tail straight from PSUM via
    tensor_scalar(min, accum_op=min) which also yields that range's per-row
    min for free, computes the remaining per-row min (macc) with a 4x-mode
    tensor_scalar min-accumulate over c[0:DSPLIT], and updates the running
    per-m min (nacc) with one full-width tensor_tensor min (2x mode).
Host: final partition-axis min, cross-core min, sqrt, means (fp64), scalar out.
"""

import numpy as np

B = 4
N = 8192
M = 8192
NCORES = 8
NSH = N // 2          # predicted points per core
NT = NSH // 128       # 32 n-tiles per core
KDIM = 16             # fp16 hi/lo split rows (4 per coord + pn pair + tn pair)
MBLK = 512            # matmul free dim
GBLK = 2048           # PSUM group (4 banks)
NG = M // GBLK        # 4 groups per n-tile
DSPLIT = 7744         # ACT drains [0, DSPLIT); DVE fused-drains [DSPLIT, M)
MSPLIT = 4096         # macc reduce split point (gates on 2nd vs 4th ACT drain)

_CACHE = {}


def _build_bass():
    from contextlib import ExitStack

    import concourse.bacc as bacc
    import concourse.mybir as mybir
    import concourse.tile as tile

    dt = mybir.dt
    amin = mybir.AluOpType.min

    nc = bacc.Bacc(
        "TRN2",
        target_bir_lowering=False,
        debug=False,
        num_devices=NCORES,
    )
    a_dram = nc.declare_dram_parameter("a", [KDIM, NSH], dt.float16, isOutput=False)
    b_dram = nc.declare_dram_parameter("b", [KDIM, M], dt.float16, isOutput=False)
    # macc partials: cols cover m in [0, MSPLIT), [MSPLIT, DSPLIT), [DSPLIT, M)
    out_macc = nc.declare_dram_parameter("out_macc", [128, 3 * NT], dt.float32, isOutput=True)
    out_nacc = nc.declare_dram_parameter("out_nacc", [128, M], dt.float16, isOutput=True)

    FMAX = 65504.0  # fp16 max; min(x, FMAX) is identity on our d2 range

    with ExitStack() as ctx:
        tc = ctx.enter_context(tile.TileContext(nc))
        const_pool = ctx.enter_context(tc.tile_pool(name="const", bufs=1))
        psum_pool = ctx.enter_context(tc.tile_pool(name="psum", bufs=2, space="PSUM"))
        c_pool = ctx.enter_context(tc.tile_pool(name="c", bufs=2))
        nacc_pool = ctx.enter_context(tc.tile_pool(name="nacc", bufs=2))
        scr_pool = ctx.enter_context(tc.tile_pool(name="scr", bufs=1))
        outp_pool = ctx.enter_context(tc.tile_pool(name="outp", bufs=1))

        a_sb = const_pool.tile([KDIM, NSH], dt.float16)
        nc.sync.dma_start(a_sb[:], a_dram[:])
        b_sb = const_pool.tile([KDIM, M], dt.float16)
        nc.sync.dma_start(b_sb[:, 0:GBLK], b_dram[:, 0:GBLK])
        nc.sync.dma_start(b_sb[:, GBLK:M], b_dram[:, GBLK:M])

        maccs = outp_pool.tile([128, 3 * NT], dt.float32)
        scr = scr_pool.tile([128, DSPLIT], dt.float16)

        nacc_prev = None
        for i in range(NT):
            first = i == 0
            if first:
                # tile 0: drains write straight into the nacc buffer (nacc_0 = c_0)
                c_i = nacc_pool.tile([128, M], dt.float16, tag="nacc")
            else:
                c_i = c_pool.tile([128, M], dt.float16, tag="c")

            ps_last = None
            for g in range(NG):
                ps = psum_pool.tile([128, GBLK], dt.float32, tag="ps")
                for q in range(GBLK // MBLK):
                    j = g * (GBLK // MBLK) + q
                    nc.tensor.matmul(
                        ps[:, q * MBLK:(q + 1) * MBLK],
                        a_sb[0:KDIM, i * 128:(i + 1) * 128],
                        b_sb[0:KDIM, j * MBLK:(j + 1) * MBLK],
                        start=True,
                        stop=True,
                    )
                lo = g * GBLK
                hi = (g + 1) * GBLK
                if hi <= DSPLIT:
                    nc.scalar.copy(c_i[:, lo:hi], ps[:])
                else:
                    # last group: ACT drains [lo, DSPLIT), DVE fused-drains the tail
                    nc.scalar.copy(c_i[:, lo:DSPLIT], ps[:, 0:DSPLIT - lo])
                    ps_last = ps

            last = i == NT - 1
            if not last:
                # DVE: fused drain + per-row min accum of the PSUM tail
                nc.vector.tensor_scalar(
                    c_i[:, DSPLIT:M], ps_last[:, DSPLIT - (M - GBLK):GBLK], FMAX, None,
                    amin, amin, accum_out=maccs[:, 2 * NT + i:2 * NT + i + 1],
                )
                if first:
                    nc.vector.tensor_scalar(
                        scr[:, 0:MSPLIT], c_i[:, 0:MSPLIT], FMAX, None, amin, amin,
                        accum_out=maccs[:, i:i + 1],
                    )
                    nc.vector.tensor_scalar(
                        scr[:, MSPLIT:DSPLIT], c_i[:, MSPLIT:DSPLIT], FMAX, None, amin, amin,
                        accum_out=maccs[:, NT + i:NT + i + 1],
                    )
                    nacc_prev = c_i
                else:
                    nacc_i = nacc_pool.tile([128, M], dt.float16, tag="nacc")
                    # TT-A only needs the first two ACT drains -> runs mid-window
                    nc.vector.tensor_tensor(nacc_i[:, 0:M // 2], c_i[:, 0:M // 2], nacc_prev[:, 0:M // 2], amin)
                    # DVE: 4x-mode per-row min, split so only part gates on the last drain
                    nc.vector.tensor_scalar(
                        scr[:, 0:MSPLIT], c_i[:, 0:MSPLIT], FMAX, None, amin, amin,
                        accum_out=maccs[:, i:i + 1],
                    )
                    nc.vector.tensor_scalar(
                        scr[:, MSPLIT:DSPLIT], c_i[:, MSPLIT:DSPLIT], FMAX, None, amin, amin,
                        accum_out=maccs[:, NT + i:NT + i + 1],
                    )
                    nc.vector.tensor_tensor(nacc_i[:, M // 2:M], c_i[:, M // 2:M], nacc_prev[:, M // 2:M], amin)
                    nacc_prev = nacc_i
            else:
                # last tile: chunk the nacc update per PSUM group so each
                # output DMA starts as soon as its m-range is final
                nacc_i = nacc_pool.tile([128, M], dt.float16, tag="nacc")
                nc.vector.tensor_scalar(
                    c_i[:, DSPLIT:M], ps_last[:, DSPLIT - (M - GBLK):GBLK], FMAX, None,
                    amin, amin, accum_out=maccs[:, 2 * NT + i:2 * NT + i + 1],
                )
                for g in range(2):
                    lo, hi = g * GBLK, (g + 1) * GBLK
                    nc.vector.tensor_tensor(nacc_i[:, lo:hi], c_i[:, lo:hi], nacc_prev[:, lo:hi], amin)
                    nc.sync.dma_start(out_nacc[:, lo:hi], nacc_i[:, lo:hi])
                nc.vector.tensor_scalar(
                    scr[:, 0:MSPLIT], c_i[:, 0:MSPLIT], FMAX, None, amin, amin,
                    accum_out=maccs[:, i:i + 1],
                )
                lo, hi = 2 * GBLK, 3 * GBLK
                nc.vector.tensor_tensor(nacc_i[:, lo:hi], c_i[:, lo:hi], nacc_prev[:, lo:hi], amin)
                nc.sync.dma_start(out_nacc[:, lo:hi], nacc_i[:, lo:hi])
                lo = (NG - 1) * GBLK
                nc.vector.tensor_tensor(nacc_i[:, lo:M], c_i[:, lo:M], nacc_prev[:, lo:M], amin)
                nc.sync.dma_start(out_nacc[:, lo:M], nacc_i[:, lo:M])
                nc.vector.tensor_scalar(
                    scr[:, MSPLIT:DSPLIT], c_i[:, MSPLIT:DSPLIT], FMAX, None, amin, amin,
                    accum_out=maccs[:, NT + i:NT + i + 1],
                )
                nc.sync.dma_start(out_macc[:], maccs[:])
                nacc_prev = nacc_i

    nc.compile()
    return nc


def _get_nc():
    if "nc" not in _CACHE:
        _CACHE["nc"] = _build_bass()
    return _CACHE["nc"]


def _split16(v):
    hi = v.astype(np.float16)
    lo = (v - hi.astype(np.float32)).astype(np.float16)
    return hi, lo


def _make_in_maps(p, t):
    in_maps = []
    for c in range(NCORES):
        b, h = divmod(c, 2)
        ps = p[b, h * NSH:(h + 1) * NSH]        # (NSH, 3)
        pn = (ps.astype(np.float64) ** 2).sum(-1).astype(np.float32)
        tb = t[b]                               # (M, 3)
        tn = (tb.astype(np.float64) ** 2).sum(-1).astype(np.float32)

        A = np.empty((KDIM, NSH), np.float16)
        Bm = np.empty((KDIM, M), np.float16)
        # rows 4d..4d+3 per coord d: lhs [ah,ah,al,al] x rhs [th,tl,th,tl]
        for d in range(3):
            ah, al = _split16(-2.0 * ps[:, d])
            th, tl = _split16(tb[:, d])
            A[4 * d + 0] = ah
            A[4 * d + 1] = ah
            A[4 * d + 2] = al
            A[4 * d + 3] = al
            Bm[4 * d + 0] = th
            Bm[4 * d + 1] = tl
            Bm[4 * d + 2] = th
            Bm[4 * d + 3] = tl
        pnh, pnl = _split16(pn)
        tnh, tnl = _split16(tn)
        A[12] = pnh
        A[13] = pnl
        A[14] = 1.0
        A[15] = 1.0
        Bm[12] = 1.0
        Bm[13] = 1.0
        Bm[14] = tnh
        Bm[15] = tnl
        in_maps.append({"a": np.ascontiguousarray(A), "b": np.ascontiguousarray(Bm)})
    return in_maps


def _combine(results):
    total = 0.0
    for b in range(B):
        pred_sum = 0.0
        tmins = []
        for h in range(2):
            r = results[2 * b + h]
            macc3 = np.asarray(r["out_macc"], np.float64)       # (128, 3*NT) d2
            macc = np.minimum(np.minimum(macc3[:, :NT], macc3[:, NT:2 * NT]), macc3[:, 2 * NT:])
            pred_sum += np.sqrt(np.maximum(macc, 0.0)).sum()
            nacc = np.asarray(r["out_nacc"], np.float32)        # (128, M) d2
            tmins.append(nacc.min(axis=0))
        mean_pred = pred_sum / N
        d2t = np.maximum(np.minimum(tmins[0], tmins[1]), 0.0).astype(np.float64)
        mean_tgt = np.sqrt(d2t).mean()
        total += (mean_pred + mean_tgt) / 2.0
    return np.asarray(total / B, dtype=np.float32)


def run_on_cores(p, t, trace=False):
    """Run the bass kernel; returns BassKernelResults."""
    from concourse.bass_utils import run_bass_kernel_spmd

    nc = _get_nc()
    in_maps = _make_in_maps(p, t)
    br = run_bass_kernel_spmd(nc, in_maps, list(range(NCORES)), trace=trace)
    return br


def kernel(predicted_points, target_points):
    p = np.asarray(predicted_points, dtype=np.float32)
    t = np.asarray(target_points, dtype=np.float32)
    assert p.shape == (B, N, 3) and t.shape == (B, M, 3)
    br = run_on_cores(p, t, trace=False)
    return _combine(br.results)
